# revision 22
# baseline (speedup 1.0000x reference)
"""ComplexAttention Trainium2 kernel (Bass/Tile, SPMD over 8 NeuronCores).

Problem: complex-valued multi-head attention (B=2, N=2048, DIM=1024, 16 heads,
head_dim 64), fp32 reference. Sharding: data-parallel over batch (2) x
tensor-parallel over head groups (4 groups x 4 heads). Each core computes
q/k/v for its 4 heads, full attention, and a partial output projection
(contraction over its 256 of the 1024 concat features); the host sums the
4 partials per batch.

v2 design (vs f32r baseline at ~491us):
- bf16 operands everywhere (PE rate identical to f32r at 1 row/cycle, but
  halves HBM traffic + SBUF, enables FWL weight loads). PSUM stays fp32.
- Pass 1 streams x ONCE (V+K+Q chains share the same 512-token x block),
  cutting x HBM traffic 3x vs the baseline's three passes.
- Pass 2 interleaves the 4 heads per 128-row m-tile so the softmax rowsum
  runs as 4 CONCURRENT col-tiled matmuls (tile_position=(0,32h), 1-col
  stationary each): ~4x cheaper than the baseline's serial ones-matmuls.
- Softmax 1/Z: rowsum [1,512] is DMA-transposed to [128,4] so the DVE
  reciprocal runs partition-parallel (0.1us vs 3.3us on [128,512]).
- Out-projection of block nb is emitted in 4-matmul chunks interleaved into
  block nb+1's attention, filling PE gaps left by EXP latency and spreading
  the y DMA.

Device data layouts (per core):
  xs    (4, 128, 16, 512) bf16  [block, d%128, (xr d/128 0..7 | xi 8..15), token]
  wq/wk (1024, 1024) bf16  cols per head h: [A_h (128) | B_h (128)],
                           A_h = [wr_h; wi_h].T cols [re|im], B_h = [-wi_h; wr_h].T
  wv    (1024, 1024) bf16  rows d, cols [A (512) | B (512)],
                           A per head [wvr_h.T | wvi_h.T], B per head [-wvi_h.T | wvr_h.T]
  wo    (1024, 1024) bf16  rows 0:512 -> y_real coeffs, 512: -> y_imag;
                           row order h*128 + c*64 + d matches AO layout
  qk_bias (128, 8) f32     per-partition bias columns [q h0..h3, k h0..h3]
  vbias (128, 512) f32     broadcast rows, cols per head [bdiff(64) | bsum(64)]
  obias (128, 2048) f32    broadcast rows [y_re 1024 | y_im 1024]; zero on g>0
Outputs: yr, yi (2048, 1024) f32 partial projections.
"""

from contextlib import ExitStack

import numpy as np
import ml_dtypes

import concourse.bacc as bacc
import concourse.mybir as mybir
import concourse.tile as tile
from concourse.bass_utils import run_bass_kernel_spmd

F32 = mybir.dt.float32
BF16 = mybir.dt.bfloat16
BFNP = ml_dtypes.bfloat16

B = 2
N = 2048
DIM = 1024
HEADS = 16
HD = 64
G = 4          # head groups (tensor-parallel factor)
HLOC = HEADS // G
SCALE = 1.0 / 8.0
P = 128
NBLK = 512     # token block (pass-1 x stream, pass-2 n block)
NB = N // NBLK # 4 blocks
MT = N // P    # 16 m-tiles

_CACHE = {}
DEBUG = False


def _build_program():
    nc = bacc.Bacc("TRN2", target_bir_lowering=False, debug=False, num_devices=8,
                   dynamic_dma_scratch_size=2048)

    xs = nc.dram_tensor("xs", [NB, P, 16, NBLK], BF16, kind="ExternalInput").ap()
    wq = nc.dram_tensor("wq", [DIM, 1024], BF16, kind="ExternalInput").ap()
    wk = nc.dram_tensor("wk", [DIM, 1024], BF16, kind="ExternalInput").ap()
    wv = nc.dram_tensor("wv", [DIM, 1024], BF16, kind="ExternalInput").ap()
    wo = nc.dram_tensor("wo", [1024, 1024], BF16, kind="ExternalInput").ap()
    qkb_d = nc.dram_tensor("qk_bias", [P, 8], F32, kind="ExternalInput").ap()
    vb_d = nc.dram_tensor("vbias", [P, 512], F32, kind="ExternalInput").ap()
    ob_d = nc.dram_tensor("obias", [P, 2048], F32, kind="ExternalInput").ap()
    yr = nc.dram_tensor("yr", [N, 1024], BF16, kind="ExternalOutput").ap()
    yi = nc.dram_tensor("yi", [N, 1024], BF16, kind="ExternalOutput").ap()

    wq_r = wq.rearrange("(t p) c -> p t c", p=P)   # [128, 8, 1024]
    wk_r = wk.rearrange("(t p) c -> p t c", p=P)
    wv_r = wv.rearrange("(t p) c -> p t c", p=P)
    wo_r = wo.rearrange("(t p) c -> p t c", p=P)

    with tile.TileContext(nc) as tc, ExitStack() as ctx:
        const = ctx.enter_context(tc.tile_pool(name="const", bufs=1))
        kvp = ctx.enter_context(tc.tile_pool(name="kv", bufs=1))

        onesc_f = const.tile([P, 1], F32)
        ones_bf = const.tile([P, 1], BF16)
        nc.vector.memset(onesc_f[:], 1.0)
        nc.vector.tensor_copy(ones_bf[:], onesc_f[:])
        ones_blk_f = const.tile([P, 128], F32)
        ones_row = const.tile([P, 128], BF16)
        nc.vector.memset(ones_blk_f[:], 1.0)
        nc.vector.tensor_copy(ones_row[:], ones_blk_f[:])
        qkb = const.tile([P, 8], F32)
        actwarm = const.tile([P, 1], F32)

        # per-block tiles so pass-2 reads only wait on the blocks they use
        Q_sb = [kvp.tile([P, HLOC, NBLK], BF16, tag=f"qsb{b}", name=f"qsb{b}")
                for b in range(NB)]                  # [comps, head, n]
        K_sb = [kvp.tile([P, HLOC, NBLK], BF16, tag=f"ksb{b}", name=f"ksb{b}")
                for b in range(NB)]                  # [comps, head, m]
        V_sb = [kvp.tile([P, 4, 512], BF16, tag=f"vsb{b}", name=f"vsb{b}")
                for b in range(NB)]                  # [m%128, mtile, (h, re|im)]

        # ---------------- pass 1: single x stream, V+K+Q per block ----------
        p1 = ExitStack()
        w1p = p1.enter_context(tc.tile_pool(name="w1", bufs=1))
        xsp = p1.enter_context(tc.tile_pool(name="xs", bufs=3))
        pmm1 = p1.enter_context(tc.tile_pool(name="p1ps", bufs=6, space="PSUM"))

        vb = w1p.tile([P, 512], F32)
        # first wave: interleave wv A-halves with x block 0 slices so each of
        # the 16 DMA queues carries exactly one ~128KB critical piece
        wv_t = [w1p.tile([P, 1024], BF16, tag=f"wv{dt}", name=f"wv{dt}")
                for dt in range(8)]
        xt0 = xsp.tile([P, 16, NBLK], BF16, tag="xs", name="xt0")
        for dt in range(8):
            nc.sync.dma_start(out=wv_t[dt][:, :512], in_=wv_r[:, dt, :512])
            nc.sync.dma_start(out=xt0[:, 2 * dt:2 * dt + 2, :],
                              in_=xs[0, :, 2 * dt:2 * dt + 2, :])
        # second issue stream on the (otherwise idle) scalar queue: wvB
        # lands before the mt0 B-half matmuls need it, wk before K(blk0)
        wk_t = [w1p.tile([P, 1024], BF16, tag=f"wk{dt}", name=f"wk{dt}")
                for dt in range(8)]
        for dt in range(8):
            nc.scalar.dma_start(out=wv_t[dt][:, 512:], in_=wv_r[:, dt, 512:])
        for dt in range(8):
            nc.scalar.dma_start(out=wk_t[dt][:], in_=wk_r[:, dt, :])
        nc.sync.dma_start(out=vb[:], in_=vb_d[:])
        nc.sync.dma_start(out=qkb[:], in_=qkb_d[:])
        xt1 = xsp.tile([P, 16, NBLK], BF16, tag="xs", name="xt1")
        nc.scalar.dma_start(out=xt1[:], in_=xs[1])
        wq_t = []
        for dt in range(8):
            wqt = w1p.tile([P, 1024], BF16, tag=f"wq{dt}", name=f"wq{dt}")
            nc.scalar.dma_start(out=wqt[:], in_=wq_r[:, dt, :])
            wq_t.append(wqt)
        # touch Ln+Exp behind the scalar-queue DMA issues: the ~1.3us ACT
        # table load lands mid-pass-1 (ACT idle) instead of on the first
        # pass-2 EXP
        nc.scalar.activation(actwarm[:], onesc_f[:],
                             mybir.ActivationFunctionType.Ln)
        nc.scalar.activation(actwarm[:], onesc_f[:],
                             mybir.ActivationFunctionType.Exp)

        def kq_chain(w_t, sb, bcol, h, xt, blk):
            ps = pmm1.tile([P, NBLK], F32, tag="mm")
            for dt in range(8):
                nc.tensor.matmul(
                    ps[:], w_t[dt][:, h * 256:h * 256 + 128],
                    xt[:, dt, :], start=(dt == 0), stop=False)
            for dt in range(8):
                nc.tensor.matmul(
                    ps[:], w_t[dt][:, h * 256 + 128:h * 256 + 256],
                    xt[:, 8 + dt, :], start=False, stop=(dt == 7))
            nc.vector.tensor_scalar_add(
                sb[blk][:, h, :], ps[:], qkb[:, bcol + h:bcol + h + 1])

        prev_q = None   # Q lags one block so the wq DMA hides
        xts = [xt0, xt1]
        for blk in range(NB):
            if blk < 2:
                xt = xts[blk]
            else:
                xt = xsp.tile([P, 16, NBLK], BF16, tag="xs")
                nc.sync.dma_start(out=xt[:], in_=xs[blk])
            for mt in range(NBLK // P):
                mtg = blk * (NBLK // P) + mt
                ps = pmm1.tile([P, 512], F32, tag="mm")
                for dt in range(8):
                    nc.tensor.matmul(
                        ps[:], xt[:, dt, mt * P:(mt + 1) * P],
                        wv_t[dt][:, :512], start=(dt == 0), stop=False)
                for dt in range(8):
                    nc.tensor.matmul(
                        ps[:], xt[:, 8 + dt, mt * P:(mt + 1) * P],
                        wv_t[dt][:, 512:], start=False, stop=(dt == 7))
                nc.vector.tensor_add(V_sb[blk][:, mt, :], ps[:], vb[:])
            for h in range(HLOC):
                kq_chain(wk_t, K_sb, 4, h, xt, blk)
            if prev_q is not None:
                for h in range(HLOC):
                    kq_chain(wq_t, Q_sb, 0, h, prev_q[0], prev_q[1])
            prev_q = (xt, blk)
        for h in range(HLOC):
            kq_chain(wq_t, Q_sb, 0, h, prev_q[0], prev_q[1])
        p1.close()

        # ---------------- pass 2: attention + partial out-projection -------
        with tc.tile_pool(name="w2", bufs=1) as w2p, \
             tc.tile_pool(name="ssp", bufs=2, space="PSUM") as ssp, \
             tc.tile_pool(name="pvp", bufs=1, space="PSUM") as pvp, \
             tc.tile_pool(name="rsp", bufs=1, space="PSUM") as rsp, \
             tc.tile_pool(name="prj", bufs=1, space="PSUM") as prj, \
             tc.tile_pool(name="epool", bufs=3) as ep, \
             tc.tile_pool(name="aop", bufs=2) as aop, \
             tc.tile_pool(name="rbp", bufs=2) as rbp, \
             tc.tile_pool(name="outp", bufs=4) as outp:
            wo_sb = w2p.tile([P, 8, 1024], BF16)
            ob = w2p.tile([P, 2048], F32)
            nc.sync.dma_start(out=wo_sb[:], in_=wo_r[:])
            nc.sync.dma_start(out=ob[:], in_=ob_d[:])

            ots = {}

            def proj_chunk(ao, pnb, c, pool=None, tag=None):
                # column-half partners (c, c+2) share one [P, 1024] ot tile
                # and a single full-row y DMA: DMA issue (~650ns/descriptor
                # on the sync queue) is the tail bottleneck, so halve the
                # descriptor count
                ns, half, ri = c // 4, (c // 2) % 2, c % 2
                ydram = yr if ri == 0 else yi
                pp = (pool or prj).tile([P, 512], F32, tag=tag or "pp",
                                        name="pp")
                for t in range(4):
                    nc.tensor.matmul(
                        pp[:], ao[:, t, ns * P:(ns + 1) * P],
                        wo_sb[:, 4 * ri + t, half * 512:(half + 1) * 512],
                        start=(t == 0), stop=(t == 3))
                key = (pnb, ns, ri)
                if key in ots:
                    ot = ots.pop(key)
                else:
                    ot = outp.tile([P, 1024], BF16, tag="ot")
                    ots[key] = ot
                nc.vector.tensor_add(
                    ot[:, half * 512:(half + 1) * 512], pp[:],
                    ob[:, ri * 1024 + half * 512:ri * 1024 + (half + 1) * 512])
                if key not in ots:
                    nc.sync.dma_start(
                        out=ydram[pnb * NBLK + ns * P:
                                  pnb * NBLK + (ns + 1) * P, :],
                        in_=ot[:])

            # chunk schedule: 12 proj chunks of the previous block spread over
            # m-tiles 4..15 of the current block (start at 4 so the prj-bank
            # WAR chain off the dance broadcasts has cleared); chunks 12-15
            # are emitted after the block's attention as boundary filler
            sched = {mt: [] for mt in range(MT)}
            for c in range(13):
                sched[3 + c].append(c)

            def pv_mm(pvs, e_t, pmt):
                for h in range(HLOC):
                    nc.tensor.matmul(
                        pvs[h][:],
                        V_sb[pmt // 4][:, pmt % 4, h * P:(h + 1) * P],
                        e_t[h][:], start=(pmt == 0), stop=(pmt == MT - 1))

            def rs_mm(rs, e_t, pmt):
                for h in range(HLOC):
                    nc.tensor.matmul(
                        rs[32 * h:32 * h + 1, :], ones_bf[:], e_t[h][:],
                        start=(pmt == 0), stop=(pmt == MT - 1),
                        tile_position=(0, 32 * h))

            def dance_rcp(rs):
                # 1/Z on the ACT engine: exp(-ln Z).  Ln and Exp share the
                # natural_log_exp_and_others table set -> no table reloads.
                # Rows other than 32h hold PSUM garbage; Ln/Exp may produce
                # inf/nan there but only rows 32h are ever read back.
                lnz = rbp.tile([P, 512], F32, tag="lnz")
                nc.scalar.activation(lnz[:], rs[:],
                                     mybir.ActivationFunctionType.Ln)
                rsi = rbp.tile([P, 512], BF16, tag="rsi")
                nc.scalar.activation(rsi[:], lnz[:],
                                     mybir.ActivationFunctionType.Exp,
                                     scale=-1.0)
                return rsi

            def dance_head(h, rsi, ao):
                # broadcast 1/Z row 32h to all partitions with a K=1 matmul
                # (row-group 32h), then scale ao in place (one PSUM operand).
                # rbr reuses the rs / prj banks (same tag -> same bank).
                pool, tag = (rsp, "rs") if h % 2 == 0 else (prj, "pp")
                rbr = pool.tile([P, 512], F32, tag=tag, name=f"rbr{h}")
                nc.tensor.matmul(rbr[:], ones_row[32 * h:32 * h + 1, :],
                                 rsi[32 * h:32 * h + 1, :],
                                 start=True, stop=True,
                                 tile_position=(32 * h, 0))
                nc.vector.tensor_mul(ao[:, h, :], ao[:, h, :], rbr[:])

            prev = None   # (ao, nb, pvs, rs)
            for nb in range(NB):
                # pvs/rs allocated lazily at first write so the psum-bank
                # rings advance in true usage order (the previous block's
                # dance reuses the same banks for its rbr broadcasts first)
                pvs = None
                rs = None
                ao = aop.tile([P, HLOC, NBLK], BF16, tag="ao")
                rsi_p = dance_rcp(prev[3]) if prev is not None else None
                e1 = None   # e tiles of mt-1 (pv pending)
                e2 = None   # e tiles of mt-2 (rowsum pending)
                for mt in range(MT):
                    e_cur = []
                    for h in range(HLOC):
                        ss = ssp.tile([P, NBLK], F32, tag="ss")
                        nc.tensor.matmul(
                            ss[:],
                            K_sb[mt // 4][:, h, (mt % 4) * P:(mt % 4 + 1) * P],
                            Q_sb[nb][:, h, :], start=True, stop=True)
                        e_ = ep.tile([P, NBLK], BF16, tag=f"e{h}")
                        nc.scalar.activation(
                            e_[:], ss[:], mybir.ActivationFunctionType.Exp,
                            scale=SCALE)
                        e_cur.append(e_)
                        if h == 1:
                            if mt == 0 and prev is not None:
                                # stage the pv psum into ao (frees pv banks
                                # early), then normalize heads 0/1; heads 2/3
                                # are emitted at the h==3 slot so the PE can
                                # run scores h2/h3 while the DVE muls drain
                                for hh in range(HLOC):
                                    nc.vector.tensor_copy(
                                        prev[0][:, hh, :], prev[2][hh][:])
                                dance_head(0, rsi_p, prev[0])
                                dance_head(1, rsi_p, prev[0])
                            if prev is not None:
                                for c in sched[mt]:
                                    proj_chunk(prev[0], prev[1], c)
                            if e1 is not None:
                                if pvs is None:
                                    pvs = [pvp.tile([P, 512], F32,
                                                    tag=f"pv{h2}",
                                                    name=f"pv{h2}")
                                           for h2 in range(HLOC)]
                                pv_mm(pvs, e1, mt - 1)
                            if e2 is not None:
                                if rs is None:
                                    rs = rsp.tile([P, 512], F32, tag="rs")
                                rs_mm(rs, e2, mt - 2)
                        if h == 3 and mt == 0 and prev is not None:
                            dance_head(2, rsi_p, prev[0])
                            dance_head(3, rsi_p, prev[0])
                    e2 = e1
                    e1 = e_cur
                pv_mm(pvs, e1, MT - 1)
                rs_mm(rs, e2, MT - 2)
                rs_mm(rs, e1, MT - 1)
                if prev is not None:
                    for c in range(13, 16):
                        proj_chunk(prev[0], prev[1], c)
                prev = (ao, nb, pvs, rs)
            # final block: dance + all 16 chunks, pipelined over 3 psum banks
            rsi_p = dance_rcp(prev[3])
            for hh in range(HLOC):
                nc.vector.tensor_copy(prev[0][:, hh, :], prev[2][hh][:])
            for hh in range(HLOC):
                dance_head(hh, rsi_p, prev[0])
            for c in range(16):
                if c % 3 == 0:
                    proj_chunk(prev[0], prev[1], c)
                else:
                    proj_chunk(prev[0], prev[1], c, pool=ssp, tag="ss")
    nc.compile()
    return nc


def _prepare_in_maps(x, wqkv_r, wqkv_i, bqkv_r, bqkv_i, wo_r, wo_i, bo_r, bo_i):
    x = np.asarray(x, np.float32)
    wqkv_r = np.asarray(wqkv_r, np.float32)
    wqkv_i = np.asarray(wqkv_i, np.float32)
    bqkv_r = np.asarray(bqkv_r, np.float32)
    bqkv_i = np.asarray(bqkv_i, np.float32)
    wo_r = np.asarray(wo_r, np.float32)
    wo_i = np.asarray(wo_i, np.float32)
    bo_r = np.asarray(bo_r, np.float32)
    bo_i = np.asarray(bo_i, np.float32)

    bdiff = bqkv_r - bqkv_i
    bsum = bqkv_r + bqkv_i

    xs_by_b = []
    for b in range(B):
        xsb = np.concatenate(
            [np.ascontiguousarray(x[b, :, :, 0].T),
             np.ascontiguousarray(x[b, :, :, 1].T)], axis=0)  # (2048 d2, 2048 m)
        xsb = xsb.reshape(16, P, NB, NBLK).transpose(2, 1, 0, 3)
        xs_by_b.append(np.ascontiguousarray(xsb).astype(BFNP))

    per_g = []
    for g in range(G):
        # q/k weights: per head, stacked-complex A/B column blocks
        def head_cols(base):
            cols = []
            for h in range(HLOC):
                rows = slice(base + g * 256 + h * HD, base + g * 256 + (h + 1) * HD)
                a = np.concatenate([wqkv_r[rows], wqkv_i[rows]], axis=0).T
                bb = np.concatenate([-wqkv_i[rows], wqkv_r[rows]], axis=0).T
                cols.append(a)
                cols.append(bb)
            return np.concatenate(cols, axis=1).astype(BFNP)  # (1024, 1024)

        wq_host = head_cols(0)
        wk_host = head_cols(DIM)

        vrows = slice(2 * DIM + g * 256, 2 * DIM + (g + 1) * 256)
        wvr = wqkv_r[vrows]   # (256, 1024), rows = (h, f)
        wvi = wqkv_i[vrows]
        A = np.empty((1024, 512), np.float32)
        Bm = np.empty((1024, 512), np.float32)
        for h in range(HLOC):
            A[:, h * P:h * P + HD] = wvr[h * HD:(h + 1) * HD].T
            A[:, h * P + HD:(h + 1) * P] = wvi[h * HD:(h + 1) * HD].T
            Bm[:, h * P:h * P + HD] = -wvi[h * HD:(h + 1) * HD].T
            Bm[:, h * P + HD:(h + 1) * P] = wvr[h * HD:(h + 1) * HD].T
        wv_host = np.concatenate([A, Bm], axis=1).astype(BFNP)

        cols_g = slice(g * 256, (g + 1) * 256)
        wotr = np.ascontiguousarray(wo_r[:, cols_g].T)   # (256 fi, 1024 fo)
        woti = np.ascontiguousarray(wo_i[:, cols_g].T)
        yr_blk = np.concatenate(
            [wotr.reshape(HLOC, HD, 1024), -woti.reshape(HLOC, HD, 1024)],
            axis=1).reshape(512, 1024)
        yi_blk = np.concatenate(
            [woti.reshape(HLOC, HD, 1024), wotr.reshape(HLOC, HD, 1024)],
            axis=1).reshape(512, 1024)
        wo_host = np.concatenate([yr_blk, yi_blk], axis=0).astype(BFNP)

        qkb = np.zeros((P, 8), np.float32)
        for h in range(HLOC):
            qrows = slice(g * 256 + h * HD, g * 256 + (h + 1) * HD)
            krows = slice(DIM + g * 256 + h * HD, DIM + g * 256 + (h + 1) * HD)
            qkb[:, h] = np.concatenate([bdiff[qrows], bsum[qrows]])
            qkb[:, 4 + h] = np.concatenate([bdiff[krows], bsum[krows]])
        vbias = np.zeros((P, 512), np.float32)
        for h in range(HLOC):
            vbias[:, h * P:h * P + HD] = bdiff[vrows][h * HD:(h + 1) * HD]
            vbias[:, h * P + HD:(h + 1) * P] = bsum[vrows][h * HD:(h + 1) * HD]
        if g == 0:
            obias = np.concatenate(
                [np.broadcast_to(bo_r - bo_i, (P, 1024)),
                 np.broadcast_to(bo_r + bo_i, (P, 1024))], axis=1).astype(np.float32)
        else:
            obias = np.zeros((P, 2048), np.float32)
        per_g.append((wq_host, wk_host, wv_host, wo_host, qkb, vbias,
                      np.ascontiguousarray(obias)))

    in_maps = []
    for core in range(8):
        b, g = divmod(core, G)
        wq_host, wk_host, wv_host, wo_host, qkb, vbias, obias = per_g[g]
        in_maps.append({
            "xs": xs_by_b[b], "wq": wq_host, "wk": wk_host, "wv": wv_host,
            "wo": wo_host, "qk_bias": qkb, "vbias": vbias, "obias": obias,
        })
    return in_maps


def _get_program():
    if "nc" not in _CACHE:
        _CACHE["nc"] = _build_program()
    return _CACHE["nc"]


def run(inputs: dict, trace: bool = False):
    """Returns (output, BassKernelResults)."""
    nc = _get_program()
    in_maps = _prepare_in_maps(**inputs)
    res = run_bass_kernel_spmd(nc, in_maps, list(range(8)), trace=trace)
    out = np.zeros((B, N, DIM, 2), np.float64)
    for core in range(8):
        b = core // G
        out[b, :, :, 0] += res.results[core]["yr"]
        out[b, :, :, 1] += res.results[core]["yi"]
    return out.astype(np.float32), res


def kernel(**inputs) -> np.ndarray:
    out, _ = run(inputs)
    return out



# revision 23
# speedup vs baseline: 1.0147x; 1.0147x over previous
"""ComplexAttention Trainium2 kernel (Bass/Tile, SPMD over 8 NeuronCores).

Problem: complex-valued multi-head attention (B=2, N=2048, DIM=1024, 16 heads,
head_dim 64), fp32 reference. Sharding: data-parallel over batch (2) x
tensor-parallel over head groups (4 groups x 4 heads). Each core computes
q/k/v for its 4 heads, full attention, and a partial output projection
(contraction over its 256 of the 1024 concat features); the host sums the
4 partials per batch.

v2 design (vs f32r baseline at ~491us):
- bf16 operands everywhere (PE rate identical to f32r at 1 row/cycle, but
  halves HBM traffic + SBUF, enables FWL weight loads). PSUM stays fp32.
- Pass 1 streams x ONCE (V+K+Q chains share the same 512-token x block),
  cutting x HBM traffic 3x vs the baseline's three passes.
- Pass 2 interleaves the 4 heads per 128-row m-tile so the softmax rowsum
  runs as 4 CONCURRENT col-tiled matmuls (tile_position=(0,32h), 1-col
  stationary each): ~4x cheaper than the baseline's serial ones-matmuls.
- Softmax 1/Z: rowsum [1,512] is DMA-transposed to [128,4] so the DVE
  reciprocal runs partition-parallel (0.1us vs 3.3us on [128,512]).
- Out-projection of block nb is emitted in 4-matmul chunks interleaved into
  block nb+1's attention, filling PE gaps left by EXP latency and spreading
  the y DMA.

Device data layouts (per core):
  xs    (4, 128, 16, 512) bf16  [block, d%128, (xr d/128 0..7 | xi 8..15), token]
  wq/wk (1024, 1024) bf16  cols per head h: [A_h (128) | B_h (128)],
                           A_h = [wr_h; wi_h].T cols [re|im], B_h = [-wi_h; wr_h].T
  wv    (1024, 1024) bf16  rows d, cols [A (512) | B (512)],
                           A per head [wvr_h.T | wvi_h.T], B per head [-wvi_h.T | wvr_h.T]
  wo    (1024, 1024) bf16  rows 0:512 -> y_real coeffs, 512: -> y_imag;
                           row order h*128 + c*64 + d matches AO layout
  qk_bias (128, 8) f32     per-partition bias columns [q h0..h3, k h0..h3]
  vbias (128, 512) f32     broadcast rows, cols per head [bdiff(64) | bsum(64)]
  obias (128, 2048) f32    broadcast rows [y_re 1024 | y_im 1024]; zero on g>0
Outputs: yr, yi (2048, 1024) f32 partial projections.
"""

from contextlib import ExitStack

import numpy as np
import ml_dtypes

import concourse.bacc as bacc
import concourse.mybir as mybir
import concourse.tile as tile
from concourse.bass_utils import run_bass_kernel_spmd

F32 = mybir.dt.float32
BF16 = mybir.dt.bfloat16
BFNP = ml_dtypes.bfloat16

B = 2
N = 2048
DIM = 1024
HEADS = 16
HD = 64
G = 4          # head groups (tensor-parallel factor)
HLOC = HEADS // G
SCALE = 1.0 / 8.0
P = 128
NBLK = 512     # token block (pass-1 x stream, pass-2 n block)
NB = N // NBLK # 4 blocks
MT = N // P    # 16 m-tiles

_CACHE = {}
DEBUG = False


def _build_program():
    nc = bacc.Bacc("TRN2", target_bir_lowering=False, debug=False, num_devices=8,
                   dynamic_dma_scratch_size=2048)

    xs = nc.dram_tensor("xs", [NB, P, 16, NBLK], BF16, kind="ExternalInput").ap()
    wq = nc.dram_tensor("wq", [DIM, 1024], BF16, kind="ExternalInput").ap()
    wk = nc.dram_tensor("wk", [DIM, 1024], BF16, kind="ExternalInput").ap()
    wv = nc.dram_tensor("wv", [DIM, 1024], BF16, kind="ExternalInput").ap()
    wo = nc.dram_tensor("wo", [1024, 1024], BF16, kind="ExternalInput").ap()
    qkb_d = nc.dram_tensor("qk_bias", [P, 8], F32, kind="ExternalInput").ap()
    vb_d = nc.dram_tensor("vbias", [P, 512], F32, kind="ExternalInput").ap()
    ob_d = nc.dram_tensor("obias", [P, 2048], F32, kind="ExternalInput").ap()
    yr = nc.dram_tensor("yr", [N, 1024], BF16, kind="ExternalOutput").ap()
    yi = nc.dram_tensor("yi", [N, 1024], BF16, kind="ExternalOutput").ap()

    wq_r = wq.rearrange("(t p) c -> p t c", p=P)   # [128, 8, 1024]
    wk_r = wk.rearrange("(t p) c -> p t c", p=P)
    wv_r = wv.rearrange("(t p) c -> p t c", p=P)
    wo_r = wo.rearrange("(t p) c -> p t c", p=P)

    with tile.TileContext(nc) as tc, ExitStack() as ctx:
        const = ctx.enter_context(tc.tile_pool(name="const", bufs=1))
        kvp = ctx.enter_context(tc.tile_pool(name="kv", bufs=1))

        onesc_f = const.tile([P, 1], F32)
        ones_bf = const.tile([P, 1], BF16)
        nc.vector.memset(onesc_f[:], 1.0)
        nc.vector.tensor_copy(ones_bf[:], onesc_f[:])
        ones_blk_f = const.tile([P, 128], F32)
        ones_row = const.tile([P, 128], BF16)
        nc.vector.memset(ones_blk_f[:], 1.0)
        nc.vector.tensor_copy(ones_row[:], ones_blk_f[:])
        qkb = const.tile([P, 8], F32)

        # per-block tiles so pass-2 reads only wait on the blocks they use
        Q_sb = [kvp.tile([P, HLOC, NBLK], BF16, tag=f"qsb{b}", name=f"qsb{b}")
                for b in range(NB)]                  # [comps, head, n]
        K_sb = [kvp.tile([P, HLOC, NBLK], BF16, tag=f"ksb{b}", name=f"ksb{b}")
                for b in range(NB)]                  # [comps, head, m]
        V_sb = [kvp.tile([P, 4, 512], BF16, tag=f"vsb{b}", name=f"vsb{b}")
                for b in range(NB)]                  # [m%128, mtile, (h, re|im)]

        # ---------------- pass 1: single x stream, V+K+Q per block ----------
        p1 = ExitStack()
        w1p = p1.enter_context(tc.tile_pool(name="w1", bufs=1))
        xsp = p1.enter_context(tc.tile_pool(name="xs", bufs=3))
        pmm1 = p1.enter_context(tc.tile_pool(name="p1ps", bufs=6, space="PSUM"))

        vb = w1p.tile([P, 512], F32)
        # first wave: interleave wv A-halves with x block 0 slices so each of
        # the 16 DMA queues carries exactly one ~128KB critical piece
        wv_t = [w1p.tile([P, 1024], BF16, tag=f"wv{dt}", name=f"wv{dt}")
                for dt in range(8)]
        xt0 = xsp.tile([P, 16, NBLK], BF16, tag="xs", name="xt0")
        for dt in range(8):
            nc.sync.dma_start(out=wv_t[dt][:, :512], in_=wv_r[:, dt, :512])
            nc.sync.dma_start(out=xt0[:, 2 * dt:2 * dt + 2, :],
                              in_=xs[0, :, 2 * dt:2 * dt + 2, :])
        wk_t = [w1p.tile([P, 1024], BF16, tag=f"wk{dt}", name=f"wk{dt}")
                for dt in range(8)]
        for dt in range(8):
            nc.sync.dma_start(out=wv_t[dt][:, 512:], in_=wv_r[:, dt, 512:])
            nc.sync.dma_start(out=wk_t[dt][:], in_=wk_r[:, dt, :])
        nc.sync.dma_start(out=vb[:], in_=vb_d[:])
        nc.sync.dma_start(out=qkb[:], in_=qkb_d[:])
        xt1 = xsp.tile([P, 16, NBLK], BF16, tag="xs", name="xt1")
        nc.sync.dma_start(out=xt1[:], in_=xs[1])
        wq_t = []
        for dt in range(8):
            wqt = w1p.tile([P, 1024], BF16, tag=f"wq{dt}", name=f"wq{dt}")
            nc.sync.dma_start(out=wqt[:], in_=wq_r[:, dt, :])
            wq_t.append(wqt)

        def kq_chain(w_t, sb, bcol, h, xt, blk):
            ps = pmm1.tile([P, NBLK], F32, tag="mm")
            for dt in range(8):
                nc.tensor.matmul(
                    ps[:], w_t[dt][:, h * 256:h * 256 + 128],
                    xt[:, dt, :], start=(dt == 0), stop=False)
            for dt in range(8):
                nc.tensor.matmul(
                    ps[:], w_t[dt][:, h * 256 + 128:h * 256 + 256],
                    xt[:, 8 + dt, :], start=False, stop=(dt == 7))
            nc.vector.tensor_scalar_add(
                sb[blk][:, h, :], ps[:], qkb[:, bcol + h:bcol + h + 1])

        prev_q = None   # Q lags one block so the wq DMA hides
        xts = [xt0, xt1]
        for blk in range(NB):
            if blk < 2:
                xt = xts[blk]
            else:
                xt = xsp.tile([P, 16, NBLK], BF16, tag="xs")
                nc.sync.dma_start(out=xt[:], in_=xs[blk])
            for mt in range(NBLK // P):
                mtg = blk * (NBLK // P) + mt
                ps = pmm1.tile([P, 512], F32, tag="mm")
                for dt in range(8):
                    nc.tensor.matmul(
                        ps[:], xt[:, dt, mt * P:(mt + 1) * P],
                        wv_t[dt][:, :512], start=(dt == 0), stop=False)
                for dt in range(8):
                    nc.tensor.matmul(
                        ps[:], xt[:, 8 + dt, mt * P:(mt + 1) * P],
                        wv_t[dt][:, 512:], start=False, stop=(dt == 7))
                nc.vector.tensor_add(V_sb[blk][:, mt, :], ps[:], vb[:])
            for h in range(HLOC):
                kq_chain(wk_t, K_sb, 4, h, xt, blk)
            if prev_q is not None:
                for h in range(HLOC):
                    kq_chain(wq_t, Q_sb, 0, h, prev_q[0], prev_q[1])
            prev_q = (xt, blk)
        for h in range(HLOC):
            kq_chain(wq_t, Q_sb, 0, h, prev_q[0], prev_q[1])
        p1.close()

        # ---------------- pass 2: attention + partial out-projection -------
        with tc.tile_pool(name="w2", bufs=1) as w2p, \
             tc.tile_pool(name="ssp", bufs=2, space="PSUM") as ssp, \
             tc.tile_pool(name="pvp", bufs=1, space="PSUM") as pvp, \
             tc.tile_pool(name="rsp", bufs=1, space="PSUM") as rsp, \
             tc.tile_pool(name="prj", bufs=1, space="PSUM") as prj, \
             tc.tile_pool(name="epool", bufs=3) as ep, \
             tc.tile_pool(name="aop", bufs=2) as aop, \
             tc.tile_pool(name="rbp", bufs=2) as rbp, \
             tc.tile_pool(name="outp", bufs=4) as outp:
            wo_sb = w2p.tile([P, 8, 1024], BF16)
            ob = w2p.tile([P, 2048], F32)
            nc.sync.dma_start(out=wo_sb[:], in_=wo_r[:])
            nc.sync.dma_start(out=ob[:], in_=ob_d[:])

            ots = {}

            def proj_chunk(ao, pnb, c, pool=None, tag=None):
                # column-half partners (c, c+2) share one [P, 1024] ot tile
                # and a single full-row y DMA: DMA issue (~650ns/descriptor
                # on the sync queue) is the tail bottleneck, so halve the
                # descriptor count
                ns, half, ri = c // 4, (c // 2) % 2, c % 2
                ydram = yr if ri == 0 else yi
                pp = (pool or prj).tile([P, 512], F32, tag=tag or "pp",
                                        name="pp")
                for t in range(4):
                    nc.tensor.matmul(
                        pp[:], ao[:, t, ns * P:(ns + 1) * P],
                        wo_sb[:, 4 * ri + t, half * 512:(half + 1) * 512],
                        start=(t == 0), stop=(t == 3))
                key = (pnb, ns, ri)
                if key in ots:
                    ot = ots.pop(key)
                else:
                    ot = outp.tile([P, 1024], BF16, tag="ot")
                    ots[key] = ot
                nc.vector.tensor_add(
                    ot[:, half * 512:(half + 1) * 512], pp[:],
                    ob[:, ri * 1024 + half * 512:ri * 1024 + (half + 1) * 512])
                if key not in ots:
                    nc.sync.dma_start(
                        out=ydram[pnb * NBLK + ns * P:
                                  pnb * NBLK + (ns + 1) * P, :],
                        in_=ot[:])

            # chunk schedule: 12 proj chunks of the previous block spread over
            # m-tiles 4..15 of the current block (start at 4 so the prj-bank
            # WAR chain off the dance broadcasts has cleared); chunks 12-15
            # are emitted after the block's attention as boundary filler
            sched = {mt: [] for mt in range(MT)}
            for c in range(13):
                sched[3 + c].append(c)

            def pv_mm(pvs, e_t, pmt):
                for h in range(HLOC):
                    nc.tensor.matmul(
                        pvs[h][:],
                        V_sb[pmt // 4][:, pmt % 4, h * P:(h + 1) * P],
                        e_t[h][:], start=(pmt == 0), stop=(pmt == MT - 1))

            def rs_mm(rs, e_t, pmt):
                for h in range(HLOC):
                    nc.tensor.matmul(
                        rs[32 * h:32 * h + 1, :], ones_bf[:], e_t[h][:],
                        start=(pmt == 0), stop=(pmt == MT - 1),
                        tile_position=(0, 32 * h))

            def dance_rcp(rs):
                # 1/Z on the ACT engine: exp(-ln Z).  Ln and Exp share the
                # natural_log_exp_and_others table set -> no table reloads.
                # Rows other than 32h hold PSUM garbage; Ln/Exp may produce
                # inf/nan there but only rows 32h are ever read back.
                lnz = rbp.tile([P, 512], F32, tag="lnz")
                nc.scalar.activation(lnz[:], rs[:],
                                     mybir.ActivationFunctionType.Ln)
                rsi = rbp.tile([P, 512], BF16, tag="rsi")
                nc.scalar.activation(rsi[:], lnz[:],
                                     mybir.ActivationFunctionType.Exp,
                                     scale=-1.0)
                return rsi

            def dance_head(h, rsi, ao):
                # broadcast 1/Z row 32h to all partitions with a K=1 matmul
                # (row-group 32h), then scale ao in place (one PSUM operand).
                # rbr reuses the rs / prj banks (same tag -> same bank).
                pool, tag = (rsp, "rs") if h % 2 == 0 else (prj, "pp")
                rbr = pool.tile([P, 512], F32, tag=tag, name=f"rbr{h}")
                nc.tensor.matmul(rbr[:], ones_row[32 * h:32 * h + 1, :],
                                 rsi[32 * h:32 * h + 1, :],
                                 start=True, stop=True,
                                 tile_position=(32 * h, 0))
                nc.vector.tensor_mul(ao[:, h, :], ao[:, h, :], rbr[:])

            prev = None   # (ao, nb, pvs, rs)
            for nb in range(NB):
                # pvs/rs allocated lazily at first write so the psum-bank
                # rings advance in true usage order (the previous block's
                # dance reuses the same banks for its rbr broadcasts first)
                pvs = None
                rs = None
                ao = aop.tile([P, HLOC, NBLK], BF16, tag="ao")
                rsi_p = dance_rcp(prev[3]) if prev is not None else None
                e1 = None   # e tiles of mt-1 (pv pending)
                e2 = None   # e tiles of mt-2 (rowsum pending)
                for mt in range(MT):
                    e_cur = []
                    for h in range(HLOC):
                        ss = ssp.tile([P, NBLK], F32, tag="ss")
                        nc.tensor.matmul(
                            ss[:],
                            K_sb[mt // 4][:, h, (mt % 4) * P:(mt % 4 + 1) * P],
                            Q_sb[nb][:, h, :], start=True, stop=True)
                        e_ = ep.tile([P, NBLK], BF16, tag=f"e{h}")
                        nc.scalar.activation(
                            e_[:], ss[:], mybir.ActivationFunctionType.Exp,
                            scale=SCALE)
                        e_cur.append(e_)
                        if h == 1:
                            if mt == 0 and prev is not None:
                                # stage the pv psum into ao (frees pv banks
                                # early), then normalize heads 0/1; heads 2/3
                                # are emitted at the h==3 slot so the PE can
                                # run scores h2/h3 while the DVE muls drain
                                for hh in range(HLOC):
                                    nc.vector.tensor_copy(
                                        prev[0][:, hh, :], prev[2][hh][:])
                                dance_head(0, rsi_p, prev[0])
                                dance_head(1, rsi_p, prev[0])
                            if prev is not None:
                                for c in sched[mt]:
                                    proj_chunk(prev[0], prev[1], c)
                            if e1 is not None:
                                if pvs is None:
                                    pvs = [pvp.tile([P, 512], F32,
                                                    tag=f"pv{h2}",
                                                    name=f"pv{h2}")
                                           for h2 in range(HLOC)]
                                pv_mm(pvs, e1, mt - 1)
                            if e2 is not None:
                                if rs is None:
                                    rs = rsp.tile([P, 512], F32, tag="rs")
                                rs_mm(rs, e2, mt - 2)
                        if h == 3 and mt == 0 and prev is not None:
                            dance_head(2, rsi_p, prev[0])
                            dance_head(3, rsi_p, prev[0])
                    e2 = e1
                    e1 = e_cur
                pv_mm(pvs, e1, MT - 1)
                rs_mm(rs, e2, MT - 2)
                rs_mm(rs, e1, MT - 1)
                if prev is not None:
                    for c in range(13, 16):
                        proj_chunk(prev[0], prev[1], c)
                prev = (ao, nb, pvs, rs)
            # final block: dance + all 16 chunks, pipelined over 3 psum banks
            rsi_p = dance_rcp(prev[3])
            for hh in range(HLOC):
                nc.vector.tensor_copy(prev[0][:, hh, :], prev[2][hh][:])
            for hh in range(HLOC):
                dance_head(hh, rsi_p, prev[0])
            for c in range(16):
                if c % 3 == 0:
                    proj_chunk(prev[0], prev[1], c)
                else:
                    proj_chunk(prev[0], prev[1], c, pool=ssp, tag="ss")
    nc.compile()
    return nc


def _prepare_in_maps(x, wqkv_r, wqkv_i, bqkv_r, bqkv_i, wo_r, wo_i, bo_r, bo_i):
    x = np.asarray(x, np.float32)
    wqkv_r = np.asarray(wqkv_r, np.float32)
    wqkv_i = np.asarray(wqkv_i, np.float32)
    bqkv_r = np.asarray(bqkv_r, np.float32)
    bqkv_i = np.asarray(bqkv_i, np.float32)
    wo_r = np.asarray(wo_r, np.float32)
    wo_i = np.asarray(wo_i, np.float32)
    bo_r = np.asarray(bo_r, np.float32)
    bo_i = np.asarray(bo_i, np.float32)

    bdiff = bqkv_r - bqkv_i
    bsum = bqkv_r + bqkv_i

    xs_by_b = []
    for b in range(B):
        xsb = np.concatenate(
            [np.ascontiguousarray(x[b, :, :, 0].T),
             np.ascontiguousarray(x[b, :, :, 1].T)], axis=0)  # (2048 d2, 2048 m)
        xsb = xsb.reshape(16, P, NB, NBLK).transpose(2, 1, 0, 3)
        xs_by_b.append(np.ascontiguousarray(xsb).astype(BFNP))

    per_g = []
    for g in range(G):
        # q/k weights: per head, stacked-complex A/B column blocks
        def head_cols(base):
            cols = []
            for h in range(HLOC):
                rows = slice(base + g * 256 + h * HD, base + g * 256 + (h + 1) * HD)
                a = np.concatenate([wqkv_r[rows], wqkv_i[rows]], axis=0).T
                bb = np.concatenate([-wqkv_i[rows], wqkv_r[rows]], axis=0).T
                cols.append(a)
                cols.append(bb)
            return np.concatenate(cols, axis=1).astype(BFNP)  # (1024, 1024)

        wq_host = head_cols(0)
        wk_host = head_cols(DIM)

        vrows = slice(2 * DIM + g * 256, 2 * DIM + (g + 1) * 256)
        wvr = wqkv_r[vrows]   # (256, 1024), rows = (h, f)
        wvi = wqkv_i[vrows]
        A = np.empty((1024, 512), np.float32)
        Bm = np.empty((1024, 512), np.float32)
        for h in range(HLOC):
            A[:, h * P:h * P + HD] = wvr[h * HD:(h + 1) * HD].T
            A[:, h * P + HD:(h + 1) * P] = wvi[h * HD:(h + 1) * HD].T
            Bm[:, h * P:h * P + HD] = -wvi[h * HD:(h + 1) * HD].T
            Bm[:, h * P + HD:(h + 1) * P] = wvr[h * HD:(h + 1) * HD].T
        wv_host = np.concatenate([A, Bm], axis=1).astype(BFNP)

        cols_g = slice(g * 256, (g + 1) * 256)
        wotr = np.ascontiguousarray(wo_r[:, cols_g].T)   # (256 fi, 1024 fo)
        woti = np.ascontiguousarray(wo_i[:, cols_g].T)
        yr_blk = np.concatenate(
            [wotr.reshape(HLOC, HD, 1024), -woti.reshape(HLOC, HD, 1024)],
            axis=1).reshape(512, 1024)
        yi_blk = np.concatenate(
            [woti.reshape(HLOC, HD, 1024), wotr.reshape(HLOC, HD, 1024)],
            axis=1).reshape(512, 1024)
        wo_host = np.concatenate([yr_blk, yi_blk], axis=0).astype(BFNP)

        qkb = np.zeros((P, 8), np.float32)
        for h in range(HLOC):
            qrows = slice(g * 256 + h * HD, g * 256 + (h + 1) * HD)
            krows = slice(DIM + g * 256 + h * HD, DIM + g * 256 + (h + 1) * HD)
            qkb[:, h] = np.concatenate([bdiff[qrows], bsum[qrows]])
            qkb[:, 4 + h] = np.concatenate([bdiff[krows], bsum[krows]])
        vbias = np.zeros((P, 512), np.float32)
        for h in range(HLOC):
            vbias[:, h * P:h * P + HD] = bdiff[vrows][h * HD:(h + 1) * HD]
            vbias[:, h * P + HD:(h + 1) * P] = bsum[vrows][h * HD:(h + 1) * HD]
        if g == 0:
            obias = np.concatenate(
                [np.broadcast_to(bo_r - bo_i, (P, 1024)),
                 np.broadcast_to(bo_r + bo_i, (P, 1024))], axis=1).astype(np.float32)
        else:
            obias = np.zeros((P, 2048), np.float32)
        per_g.append((wq_host, wk_host, wv_host, wo_host, qkb, vbias,
                      np.ascontiguousarray(obias)))

    in_maps = []
    for core in range(8):
        b, g = divmod(core, G)
        wq_host, wk_host, wv_host, wo_host, qkb, vbias, obias = per_g[g]
        in_maps.append({
            "xs": xs_by_b[b], "wq": wq_host, "wk": wk_host, "wv": wv_host,
            "wo": wo_host, "qk_bias": qkb, "vbias": vbias, "obias": obias,
        })
    return in_maps


def _get_program():
    if "nc" not in _CACHE:
        _CACHE["nc"] = _build_program()
    return _CACHE["nc"]


def run(inputs: dict, trace: bool = False):
    """Returns (output, BassKernelResults)."""
    nc = _get_program()
    in_maps = _prepare_in_maps(**inputs)
    res = run_bass_kernel_spmd(nc, in_maps, list(range(8)), trace=trace)
    out = np.zeros((B, N, DIM, 2), np.float64)
    for core in range(8):
        b = core // G
        out[b, :, :, 0] += res.results[core]["yr"]
        out[b, :, :, 1] += res.results[core]["yi"]
    return out.astype(np.float32), res


def kernel(**inputs) -> np.ndarray:
    out, _ = run(inputs)
    return out



# revision 25
# speedup vs baseline: 1.0215x; 1.0067x over previous
"""ComplexAttention Trainium2 kernel (Bass/Tile, SPMD over 8 NeuronCores).

Problem: complex-valued multi-head attention (B=2, N=2048, DIM=1024, 16 heads,
head_dim 64), fp32 reference. Sharding: data-parallel over batch (2) x
tensor-parallel over head groups (4 groups x 4 heads). Each core computes
q/k/v for its 4 heads, full attention, and a partial output projection
(contraction over its 256 of the 1024 concat features); the host sums the
4 bf16 partials per batch in fp32.

v3 design (v2 at ~521us -> ~427us on the same device):
- bf16 operands everywhere (PE 1 row/cycle, halves HBM+SBUF, FWL weight
  loads). PSUM stays fp32. y partials stored bf16 (halves store traffic).
- Pass 1 streams x ONCE (V+K+Q chains share the same 512-token x block);
  wk DMAs interleave with the wv B-halves so K(blk0) is never load-stalled.
- Pass 2 interleaves the 4 heads per 128-row m-tile; softmax rowsum runs as
  4 CONCURRENT col-tiled matmuls (tile_position=(0,32h), 1-col stationary).
  The rowsum accumulation trails pv by one m-tile (epool bufs=3) so the
  rs-bank WAR chain off the previous dance never blocks the PE queue.
- Softmax 1/Z ("dance", the former bottleneck): computed as exp(-ln Z) on
  the ACT engine (Ln+Exp share one table set, preloaded at kernel start),
  broadcast to all partitions with K=1 matmuls from partition row 32h
  (tile_position=(32h,0)) into the recycled rs/prj psum banks, then one
  DVE mul scales the staged ao in place.  No SB->SB DMAs, no gpsimd
  broadcast, no 4us DVE reciprocal; block-boundary PE stalls drop from
  ~9us to ~2us and the HAM clock stays warm.
- Out-projection of block nb is emitted in 4-matmul chunks interleaved into
  block nb+1's attention (m-tiles 3..15 + 3 at the block end as dance
  filler). Column-half partner chunks share one [128,1024] ot tile and a
  single full-row y DMA (DMA issue at ~650ns/descriptor is the tail
  bottleneck); the final block's stores alternate sync/scalar queues.
- K/Q/V live in per-block SBUF tiles so first-pass-2 scores only wait on
  the blocks they read.

Device data layouts (per core):
  xs    (4, 128, 16, 512) bf16  [block, d%128, (xr d/128 0..7 | xi 8..15), token]
  wq/wk (1024, 1024) bf16  cols per head h: [A_h (128) | B_h (128)],
                           A_h = [wr_h; wi_h].T cols [re|im], B_h = [-wi_h; wr_h].T
  wv    (1024, 1024) bf16  rows d, cols [A (512) | B (512)],
                           A per head [wvr_h.T | wvi_h.T], B per head [-wvi_h.T | wvr_h.T]
  wo    (1024, 1024) bf16  rows 0:512 -> y_real coeffs, 512: -> y_imag;
                           row order h*128 + c*64 + d matches AO layout
  qk_bias (128, 8) f32     per-partition bias columns [q h0..h3, k h0..h3]
  vbias (128, 512) f32     broadcast rows, cols per head [bdiff(64) | bsum(64)]
  obias (128, 2048) f32    broadcast rows [y_re 1024 | y_im 1024]; zero on g>0
Outputs: yr, yi (2048, 1024) bf16 partial projections (host sums in fp32).
"""

from contextlib import ExitStack

import numpy as np
import ml_dtypes

import concourse.bacc as bacc
import concourse.mybir as mybir
import concourse.tile as tile
from concourse.bass_utils import run_bass_kernel_spmd

F32 = mybir.dt.float32
BF16 = mybir.dt.bfloat16
BFNP = ml_dtypes.bfloat16

B = 2
N = 2048
DIM = 1024
HEADS = 16
HD = 64
G = 4          # head groups (tensor-parallel factor)
HLOC = HEADS // G
SCALE = 1.0 / 8.0
P = 128
NBLK = 512     # token block (pass-1 x stream, pass-2 n block)
NB = N // NBLK # 4 blocks
MT = N // P    # 16 m-tiles

_CACHE = {}
DEBUG = False


def _build_program():
    nc = bacc.Bacc("TRN2", target_bir_lowering=False, debug=False, num_devices=8,
                   dynamic_dma_scratch_size=2048)

    xs = nc.dram_tensor("xs", [NB, P, 16, NBLK], BF16, kind="ExternalInput").ap()
    wq = nc.dram_tensor("wq", [DIM, 1024], BF16, kind="ExternalInput").ap()
    wk = nc.dram_tensor("wk", [DIM, 1024], BF16, kind="ExternalInput").ap()
    wv = nc.dram_tensor("wv", [DIM, 1024], BF16, kind="ExternalInput").ap()
    wo = nc.dram_tensor("wo", [1024, 1024], BF16, kind="ExternalInput").ap()
    qkb_d = nc.dram_tensor("qk_bias", [P, 8], F32, kind="ExternalInput").ap()
    vb_d = nc.dram_tensor("vbias", [P, 512], F32, kind="ExternalInput").ap()
    ob_d = nc.dram_tensor("obias", [P, 2048], F32, kind="ExternalInput").ap()
    yr = nc.dram_tensor("yr", [N, 1024], BF16, kind="ExternalOutput").ap()
    yi = nc.dram_tensor("yi", [N, 1024], BF16, kind="ExternalOutput").ap()

    wq_r = wq.rearrange("(t p) c -> p t c", p=P)   # [128, 8, 1024]
    wk_r = wk.rearrange("(t p) c -> p t c", p=P)
    wv_r = wv.rearrange("(t p) c -> p t c", p=P)
    wo_r = wo.rearrange("(t p) c -> p t c", p=P)

    with tile.TileContext(nc) as tc, ExitStack() as ctx:
        const = ctx.enter_context(tc.tile_pool(name="const", bufs=1))
        kvp = ctx.enter_context(tc.tile_pool(name="kv", bufs=1))

        onesc_f = const.tile([P, 1], F32)
        ones_bf = const.tile([P, 1], BF16)
        nc.vector.memset(onesc_f[:], 1.0)
        nc.vector.tensor_copy(ones_bf[:], onesc_f[:])
        ones_blk_f = const.tile([P, 128], F32)
        ones_row = const.tile([P, 128], BF16)
        nc.vector.memset(ones_blk_f[:], 1.0)
        nc.vector.tensor_copy(ones_row[:], ones_blk_f[:])
        qkb = const.tile([P, 8], F32)
        # touch Ln+Exp early: the ~1.3us ACT table load runs during pass 1
        # (ACT idle) instead of on the first pass-2 EXP
        actwarm = const.tile([P, 1], F32)
        nc.scalar.activation(actwarm[:], onesc_f[:],
                             mybir.ActivationFunctionType.Ln)
        nc.scalar.activation(actwarm[:], onesc_f[:],
                             mybir.ActivationFunctionType.Exp)

        # per-block tiles so pass-2 reads only wait on the blocks they use
        Q_sb = [kvp.tile([P, HLOC, NBLK], BF16, tag=f"qsb{b}", name=f"qsb{b}")
                for b in range(NB)]                  # [comps, head, n]
        K_sb = [kvp.tile([P, HLOC, NBLK], BF16, tag=f"ksb{b}", name=f"ksb{b}")
                for b in range(NB)]                  # [comps, head, m]
        V_sb = [kvp.tile([P, 4, 512], BF16, tag=f"vsb{b}", name=f"vsb{b}")
                for b in range(NB)]                  # [m%128, mtile, (h, re|im)]

        # ---------------- pass 1: single x stream, V+K+Q per block ----------
        p1 = ExitStack()
        w1p = p1.enter_context(tc.tile_pool(name="w1", bufs=1))
        xsp = p1.enter_context(tc.tile_pool(name="xs", bufs=3))
        pmm1 = p1.enter_context(tc.tile_pool(name="p1ps", bufs=6, space="PSUM"))

        vb = w1p.tile([P, 512], F32)
        # first wave: interleave wv A-halves with x block 0 slices so each of
        # the 16 DMA queues carries exactly one ~128KB critical piece
        wv_t = [w1p.tile([P, 1024], BF16, tag=f"wv{dt}", name=f"wv{dt}")
                for dt in range(8)]
        xt0 = xsp.tile([P, 16, NBLK], BF16, tag="xs", name="xt0")
        for dt in range(8):
            nc.sync.dma_start(out=wv_t[dt][:, :512], in_=wv_r[:, dt, :512])
            nc.sync.dma_start(out=xt0[:, 2 * dt:2 * dt + 2, :],
                              in_=xs[0, :, 2 * dt:2 * dt + 2, :])
        wk_t = [w1p.tile([P, 1024], BF16, tag=f"wk{dt}", name=f"wk{dt}")
                for dt in range(8)]
        for dt in range(8):
            nc.sync.dma_start(out=wv_t[dt][:, 512:], in_=wv_r[:, dt, 512:])
            nc.sync.dma_start(out=wk_t[dt][:], in_=wk_r[:, dt, :])
        nc.sync.dma_start(out=vb[:], in_=vb_d[:])
        nc.sync.dma_start(out=qkb[:], in_=qkb_d[:])
        xt1 = xsp.tile([P, 16, NBLK], BF16, tag="xs", name="xt1")
        nc.sync.dma_start(out=xt1[:], in_=xs[1])
        wq_t = []
        for dt in range(8):
            wqt = w1p.tile([P, 1024], BF16, tag=f"wq{dt}", name=f"wq{dt}")
            nc.sync.dma_start(out=wqt[:], in_=wq_r[:, dt, :])
            wq_t.append(wqt)

        def kq_chain(w_t, sb, bcol, h, xt, blk):
            ps = pmm1.tile([P, NBLK], F32, tag="mm")
            for dt in range(8):
                nc.tensor.matmul(
                    ps[:], w_t[dt][:, h * 256:h * 256 + 128],
                    xt[:, dt, :], start=(dt == 0), stop=False)
            for dt in range(8):
                nc.tensor.matmul(
                    ps[:], w_t[dt][:, h * 256 + 128:h * 256 + 256],
                    xt[:, 8 + dt, :], start=False, stop=(dt == 7))
            nc.vector.tensor_scalar_add(
                sb[blk][:, h, :], ps[:], qkb[:, bcol + h:bcol + h + 1])

        prev_q = None   # Q lags one block so the wq DMA hides
        xts = [xt0, xt1]
        for blk in range(NB):
            if blk < 2:
                xt = xts[blk]
            else:
                xt = xsp.tile([P, 16, NBLK], BF16, tag="xs")
                nc.sync.dma_start(out=xt[:], in_=xs[blk])
            for mt in range(NBLK // P):
                mtg = blk * (NBLK // P) + mt
                ps = pmm1.tile([P, 512], F32, tag="mm")
                for dt in range(8):
                    nc.tensor.matmul(
                        ps[:], xt[:, dt, mt * P:(mt + 1) * P],
                        wv_t[dt][:, :512], start=(dt == 0), stop=False)
                for dt in range(8):
                    nc.tensor.matmul(
                        ps[:], xt[:, 8 + dt, mt * P:(mt + 1) * P],
                        wv_t[dt][:, 512:], start=False, stop=(dt == 7))
                nc.vector.tensor_add(V_sb[blk][:, mt, :], ps[:], vb[:])
            for h in range(HLOC):
                kq_chain(wk_t, K_sb, 4, h, xt, blk)
            if prev_q is not None:
                for h in range(HLOC):
                    kq_chain(wq_t, Q_sb, 0, h, prev_q[0], prev_q[1])
            prev_q = (xt, blk)
        for h in range(HLOC):
            kq_chain(wq_t, Q_sb, 0, h, prev_q[0], prev_q[1])
        p1.close()

        # ---------------- pass 2: attention + partial out-projection -------
        with tc.tile_pool(name="w2", bufs=1) as w2p, \
             tc.tile_pool(name="ssp", bufs=2, space="PSUM") as ssp, \
             tc.tile_pool(name="pvp", bufs=1, space="PSUM") as pvp, \
             tc.tile_pool(name="rsp", bufs=1, space="PSUM") as rsp, \
             tc.tile_pool(name="prj", bufs=1, space="PSUM") as prj, \
             tc.tile_pool(name="epool", bufs=3) as ep, \
             tc.tile_pool(name="aop", bufs=2) as aop, \
             tc.tile_pool(name="rbp", bufs=2) as rbp, \
             tc.tile_pool(name="outp", bufs=4) as outp:
            wo_sb = w2p.tile([P, 8, 1024], BF16)
            ob = w2p.tile([P, 2048], F32)
            nc.sync.dma_start(out=wo_sb[:], in_=wo_r[:])
            nc.sync.dma_start(out=ob[:], in_=ob_d[:])

            ots = {}

            def proj_chunk(ao, pnb, c, pool=None, tag=None, eng=None):
                # column-half partners (c, c+2) share one [P, 1024] ot tile
                # and a single full-row y DMA: DMA issue (~650ns/descriptor
                # on the sync queue) is the tail bottleneck, so halve the
                # descriptor count
                ns, half, ri = c // 4, (c // 2) % 2, c % 2
                ydram = yr if ri == 0 else yi
                pp = (pool or prj).tile([P, 512], F32, tag=tag or "pp",
                                        name="pp")
                for t in range(4):
                    nc.tensor.matmul(
                        pp[:], ao[:, t, ns * P:(ns + 1) * P],
                        wo_sb[:, 4 * ri + t, half * 512:(half + 1) * 512],
                        start=(t == 0), stop=(t == 3))
                key = (pnb, ns, ri)
                if key in ots:
                    ot = ots.pop(key)
                else:
                    ot = outp.tile([P, 1024], BF16, tag="ot")
                    ots[key] = ot
                nc.vector.tensor_add(
                    ot[:, half * 512:(half + 1) * 512], pp[:],
                    ob[:, ri * 1024 + half * 512:ri * 1024 + (half + 1) * 512])
                if key not in ots:
                    (eng or nc.sync).dma_start(
                        out=ydram[pnb * NBLK + ns * P:
                                  pnb * NBLK + (ns + 1) * P, :],
                        in_=ot[:])

            # chunk schedule: 12 proj chunks of the previous block spread over
            # m-tiles 4..15 of the current block (start at 4 so the prj-bank
            # WAR chain off the dance broadcasts has cleared); chunks 12-15
            # are emitted after the block's attention as boundary filler
            sched = {mt: [] for mt in range(MT)}
            for c in range(13):
                sched[3 + c].append(c)

            def pv_mm(pvs, e_t, pmt):
                for h in range(HLOC):
                    nc.tensor.matmul(
                        pvs[h][:],
                        V_sb[pmt // 4][:, pmt % 4, h * P:(h + 1) * P],
                        e_t[h][:], start=(pmt == 0), stop=(pmt == MT - 1))

            def rs_mm(rs, e_t, pmt):
                for h in range(HLOC):
                    nc.tensor.matmul(
                        rs[32 * h:32 * h + 1, :], ones_bf[:], e_t[h][:],
                        start=(pmt == 0), stop=(pmt == MT - 1),
                        tile_position=(0, 32 * h))

            def dance_rcp(rs):
                # 1/Z on the ACT engine: exp(-ln Z).  Ln and Exp share the
                # natural_log_exp_and_others table set -> no table reloads.
                # Rows other than 32h hold PSUM garbage; Ln/Exp may produce
                # inf/nan there but only rows 32h are ever read back.
                lnz = rbp.tile([P, 512], F32, tag="lnz")
                nc.scalar.activation(lnz[:], rs[:],
                                     mybir.ActivationFunctionType.Ln)
                rsi = rbp.tile([P, 512], BF16, tag="rsi")
                nc.scalar.activation(rsi[:], lnz[:],
                                     mybir.ActivationFunctionType.Exp,
                                     scale=-1.0)
                return rsi

            def dance_head(h, rsi, ao):
                # broadcast 1/Z row 32h to all partitions with a K=1 matmul
                # (row-group 32h), then scale ao in place (one PSUM operand).
                # rbr reuses the rs / prj banks (same tag -> same bank).
                pool, tag = (rsp, "rs") if h % 2 == 0 else (prj, "pp")
                rbr = pool.tile([P, 512], F32, tag=tag, name=f"rbr{h}")
                nc.tensor.matmul(rbr[:], ones_row[32 * h:32 * h + 1, :],
                                 rsi[32 * h:32 * h + 1, :],
                                 start=True, stop=True,
                                 tile_position=(32 * h, 0))
                nc.vector.tensor_mul(ao[:, h, :], ao[:, h, :], rbr[:])

            prev = None   # (ao, nb, pvs, rs)
            for nb in range(NB):
                # pvs/rs allocated lazily at first write so the psum-bank
                # rings advance in true usage order (the previous block's
                # dance reuses the same banks for its rbr broadcasts first)
                pvs = None
                rs = None
                ao = aop.tile([P, HLOC, NBLK], BF16, tag="ao")
                rsi_p = dance_rcp(prev[3]) if prev is not None else None
                e1 = None   # e tiles of mt-1 (pv pending)
                e2 = None   # e tiles of mt-2 (rowsum pending)
                for mt in range(MT):
                    e_cur = []
                    for h in range(HLOC):
                        ss = ssp.tile([P, NBLK], F32, tag="ss")
                        nc.tensor.matmul(
                            ss[:],
                            K_sb[mt // 4][:, h, (mt % 4) * P:(mt % 4 + 1) * P],
                            Q_sb[nb][:, h, :], start=True, stop=True)
                        e_ = ep.tile([P, NBLK], BF16, tag=f"e{h}")
                        nc.scalar.activation(
                            e_[:], ss[:], mybir.ActivationFunctionType.Exp,
                            scale=SCALE)
                        e_cur.append(e_)
                        if h == 1:
                            if mt == 0 and prev is not None:
                                # stage the pv psum into ao (frees pv banks
                                # early), then normalize heads 0/1; heads 2/3
                                # are emitted at the h==3 slot so the PE can
                                # run scores h2/h3 while the DVE muls drain
                                for hh in range(HLOC):
                                    nc.vector.tensor_copy(
                                        prev[0][:, hh, :], prev[2][hh][:])
                                dance_head(0, rsi_p, prev[0])
                                dance_head(1, rsi_p, prev[0])
                            if prev is not None:
                                for c in sched[mt]:
                                    proj_chunk(prev[0], prev[1], c)
                            if e1 is not None:
                                if pvs is None:
                                    pvs = [pvp.tile([P, 512], F32,
                                                    tag=f"pv{h2}",
                                                    name=f"pv{h2}")
                                           for h2 in range(HLOC)]
                                pv_mm(pvs, e1, mt - 1)
                            if e2 is not None:
                                if rs is None:
                                    rs = rsp.tile([P, 512], F32, tag="rs")
                                rs_mm(rs, e2, mt - 2)
                        if h == 3 and mt == 0 and prev is not None:
                            dance_head(2, rsi_p, prev[0])
                            dance_head(3, rsi_p, prev[0])
                    e2 = e1
                    e1 = e_cur
                pv_mm(pvs, e1, MT - 1)
                rs_mm(rs, e2, MT - 2)
                rs_mm(rs, e1, MT - 1)
                if prev is not None:
                    for c in range(13, 16):
                        proj_chunk(prev[0], prev[1], c)
                prev = (ao, nb, pvs, rs)
            # final block: dance + all 16 chunks, pipelined over 3 psum banks
            rsi_p = dance_rcp(prev[3])
            for hh in range(HLOC):
                nc.vector.tensor_copy(prev[0][:, hh, :], prev[2][hh][:])
            for hh in range(HLOC):
                dance_head(hh, rsi_p, prev[0])
            for c in range(16):
                eng = nc.scalar if (c // 4) % 2 else nc.sync
                if c % 3 == 0:
                    proj_chunk(prev[0], prev[1], c, eng=eng)
                else:
                    proj_chunk(prev[0], prev[1], c, pool=ssp, tag="ss",
                               eng=eng)
    nc.compile()
    return nc


def _prepare_in_maps(x, wqkv_r, wqkv_i, bqkv_r, bqkv_i, wo_r, wo_i, bo_r, bo_i):
    x = np.asarray(x, np.float32)
    wqkv_r = np.asarray(wqkv_r, np.float32)
    wqkv_i = np.asarray(wqkv_i, np.float32)
    bqkv_r = np.asarray(bqkv_r, np.float32)
    bqkv_i = np.asarray(bqkv_i, np.float32)
    wo_r = np.asarray(wo_r, np.float32)
    wo_i = np.asarray(wo_i, np.float32)
    bo_r = np.asarray(bo_r, np.float32)
    bo_i = np.asarray(bo_i, np.float32)

    bdiff = bqkv_r - bqkv_i
    bsum = bqkv_r + bqkv_i

    xs_by_b = []
    for b in range(B):
        xsb = np.concatenate(
            [np.ascontiguousarray(x[b, :, :, 0].T),
             np.ascontiguousarray(x[b, :, :, 1].T)], axis=0)  # (2048 d2, 2048 m)
        xsb = xsb.reshape(16, P, NB, NBLK).transpose(2, 1, 0, 3)
        xs_by_b.append(np.ascontiguousarray(xsb).astype(BFNP))

    per_g = []
    for g in range(G):
        # q/k weights: per head, stacked-complex A/B column blocks
        def head_cols(base):
            cols = []
            for h in range(HLOC):
                rows = slice(base + g * 256 + h * HD, base + g * 256 + (h + 1) * HD)
                a = np.concatenate([wqkv_r[rows], wqkv_i[rows]], axis=0).T
                bb = np.concatenate([-wqkv_i[rows], wqkv_r[rows]], axis=0).T
                cols.append(a)
                cols.append(bb)
            return np.concatenate(cols, axis=1).astype(BFNP)  # (1024, 1024)

        wq_host = head_cols(0)
        wk_host = head_cols(DIM)

        vrows = slice(2 * DIM + g * 256, 2 * DIM + (g + 1) * 256)
        wvr = wqkv_r[vrows]   # (256, 1024), rows = (h, f)
        wvi = wqkv_i[vrows]
        A = np.empty((1024, 512), np.float32)
        Bm = np.empty((1024, 512), np.float32)
        for h in range(HLOC):
            A[:, h * P:h * P + HD] = wvr[h * HD:(h + 1) * HD].T
            A[:, h * P + HD:(h + 1) * P] = wvi[h * HD:(h + 1) * HD].T
            Bm[:, h * P:h * P + HD] = -wvi[h * HD:(h + 1) * HD].T
            Bm[:, h * P + HD:(h + 1) * P] = wvr[h * HD:(h + 1) * HD].T
        wv_host = np.concatenate([A, Bm], axis=1).astype(BFNP)

        cols_g = slice(g * 256, (g + 1) * 256)
        wotr = np.ascontiguousarray(wo_r[:, cols_g].T)   # (256 fi, 1024 fo)
        woti = np.ascontiguousarray(wo_i[:, cols_g].T)
        yr_blk = np.concatenate(
            [wotr.reshape(HLOC, HD, 1024), -woti.reshape(HLOC, HD, 1024)],
            axis=1).reshape(512, 1024)
        yi_blk = np.concatenate(
            [woti.reshape(HLOC, HD, 1024), wotr.reshape(HLOC, HD, 1024)],
            axis=1).reshape(512, 1024)
        wo_host = np.concatenate([yr_blk, yi_blk], axis=0).astype(BFNP)

        qkb = np.zeros((P, 8), np.float32)
        for h in range(HLOC):
            qrows = slice(g * 256 + h * HD, g * 256 + (h + 1) * HD)
            krows = slice(DIM + g * 256 + h * HD, DIM + g * 256 + (h + 1) * HD)
            qkb[:, h] = np.concatenate([bdiff[qrows], bsum[qrows]])
            qkb[:, 4 + h] = np.concatenate([bdiff[krows], bsum[krows]])
        vbias = np.zeros((P, 512), np.float32)
        for h in range(HLOC):
            vbias[:, h * P:h * P + HD] = bdiff[vrows][h * HD:(h + 1) * HD]
            vbias[:, h * P + HD:(h + 1) * P] = bsum[vrows][h * HD:(h + 1) * HD]
        if g == 0:
            obias = np.concatenate(
                [np.broadcast_to(bo_r - bo_i, (P, 1024)),
                 np.broadcast_to(bo_r + bo_i, (P, 1024))], axis=1).astype(np.float32)
        else:
            obias = np.zeros((P, 2048), np.float32)
        per_g.append((wq_host, wk_host, wv_host, wo_host, qkb, vbias,
                      np.ascontiguousarray(obias)))

    in_maps = []
    for core in range(8):
        b, g = divmod(core, G)
        wq_host, wk_host, wv_host, wo_host, qkb, vbias, obias = per_g[g]
        in_maps.append({
            "xs": xs_by_b[b], "wq": wq_host, "wk": wk_host, "wv": wv_host,
            "wo": wo_host, "qk_bias": qkb, "vbias": vbias, "obias": obias,
        })
    return in_maps


def _get_program():
    if "nc" not in _CACHE:
        _CACHE["nc"] = _build_program()
    return _CACHE["nc"]


def run(inputs: dict, trace: bool = False):
    """Returns (output, BassKernelResults)."""
    nc = _get_program()
    in_maps = _prepare_in_maps(**inputs)
    res = run_bass_kernel_spmd(nc, in_maps, list(range(8)), trace=trace)
    out = np.zeros((B, N, DIM, 2), np.float64)
    for core in range(8):
        b = core // G
        out[b, :, :, 0] += res.results[core]["yr"]
        out[b, :, :, 1] += res.results[core]["yi"]
    return out.astype(np.float32), res


def kernel(**inputs) -> np.ndarray:
    out, _ = run(inputs)
    return out

